# revision 26
# baseline (speedup 1.0000x reference)
"""Multi-head attention forward (B=4,T=2048,C=1024,H=16 causal) on 8 TRN2
NeuronCores via Bass/Tile.

Sharding: batch x head-group. Core c handles batch b=c//2 and heads
[g*8,(g+1)*8) where g=c%2. Each core computes its QKV projections
(column-sharded), causal attention for its 8 heads, and a row-sharded
partial of the output projection. The host sums the two partials per
batch and adds the bias.

Device layouts (T=2048, C=1024, HD=512 local head-dims):
  - scores are computed transposed (k on partitions, q free) so the
    softmax denominator falls out of the attn@V matmul via a ones-column
    appended to V ("vext"), and causal masking is a PSUM seed matmul of
    a constant bias tile (start=True) that the scores accumulate onto.
  - normalization: reciprocal of the sums row, gpsimd partition-broadcast,
    one vector multiply. No transposes anywhere on device.
"""
import sys
sys.path.insert(0, '/opt/trn_rl_repo')

import numpy as np
import ml_dtypes

B, T, C, H, D = 4, 2048, 1024, 16, 64
HPC, HD = 8, 512            # heads per core, local head-dim total
NEG = -30000.0

_CACHE = {}


def _build():
    from contextlib import ExitStack
    import concourse.bacc as bacc
    import concourse.tile as tile
    from concourse import mybir

    f32 = mybir.dt.float32
    f32r = mybir.dt.float32r
    bf16 = mybir.dt.bfloat16
    EXP = mybir.ActivationFunctionType.Exp

    nc = bacc.Bacc("TRN2", target_bir_lowering=False, debug=False, num_devices=1)

    xq_d = nc.dram_tensor("xqT", [C, T], bf16, kind="ExternalInput").ap()
    xk_d = nc.dram_tensor("xkT", [C, T], bf16, kind="ExternalInput").ap()
    xv_d = nc.dram_tensor("xvT", [C, T], bf16, kind="ExternalInput").ap()
    wq_d = nc.dram_tensor("wq", [C, HD], bf16, kind="ExternalInput").ap()
    wk_d = nc.dram_tensor("wk", [C, HD], bf16, kind="ExternalInput").ap()
    wv_d = nc.dram_tensor("wv", [C, HD], bf16, kind="ExternalInput").ap()
    wo_d = nc.dram_tensor("wo", [HD, C], bf16, kind="ExternalInput").ap()
    cd_d = nc.dram_tensor("cdiag", [128, 128], bf16, kind="ExternalInput").ap()
    y_d = nc.dram_tensor("y", [T, C], f32, kind="ExternalOutput").ap()

    with tile.TileContext(nc) as tc, ExitStack() as ctx:
        pw = ctx.enter_context(tc.tile_pool(name="pw", bufs=1))
        pqts = ctx.enter_context(tc.tile_pool(name="pqts", bufs=3))
        pkts = ctx.enter_context(tc.tile_pool(name="pkts", bufs=4))
        pvext = ctx.enter_context(tc.tile_pool(name="pvext", bufs=16))
        pctxn = ctx.enter_context(tc.tile_pool(name="pctxn", bufs=2))
        px = ctx.enter_context(tc.tile_pool(name="px", bufs=8))
        pex = ctx.enter_context(tc.tile_pool(name="pex", bufs=12))
        pr = ctx.enter_context(tc.tile_pool(name="pr", bufs=4))
        pys = ctx.enter_context(tc.tile_pool(name="pys", bufs=3))
        pps = ctx.enter_context(tc.tile_pool(name="pps", bufs=8, space="PSUM"))

        # ---- constants + resident weights (wq first so proj_q(0) can
        # start compute while the rest stream in)
        cd = pw.tile([128, 128], bf16, tag="cd")
        nc.sync.dma_start(cd[:], cd_d[:])
        wq_s = pw.tile([128, 8, HD], bf16, tag="wq")
        wk_s = pw.tile([128, 8, HD], bf16, tag="wk")
        wv_s = pw.tile([128, 8, HD], bf16, tag="wv")
        wo_s = pw.tile([128, 4, C], bf16, tag="wo")

        kts = [None] * 4     # kT window tiles [128, 4, 512]
        vext = [None] * 16   # vext chunk tiles [128, 8, 65] bf16

        def load_x2(x_src, cp, t4):
            """One 512KB DMA: c-tiles 2cp,2cp+1 of window t4 -> [128,2,512]."""
            x2 = px.tile([128, 2, 512], bf16, tag="x", name="x2")
            nc.sync.dma_start(
                x2[:],
                x_src[2 * cp * 128:(2 * cp + 2) * 128,
                      t4 * 512:(t4 + 1) * 512].rearrange(
                          "(two p) t -> p two t", p=128))
            return x2

        def proj_qk(w_s, x_src, t4, tag, w_src=None):
            """qT/kT window: out[pair-row, hp, t] for t in window t4.
            w_src streams the weight c-tiles just-in-time (first use)."""
            ps = [pps.tile([128, 512], f32, tag="ps", name=f"ps{i}") for i in range(4)]
            for cp in range(4):
                if w_src is not None:
                    for h2 in range(2):
                        ct = 2 * cp + h2
                        nc.sync.dma_start(
                            w_s[:, ct, :],
                            w_src[ct * 128:(ct + 1) * 128, :])
                x2 = load_x2(x_src, cp, t4)
                for half in range(2):
                    ct = 2 * cp + half
                    for j in range(4):
                        nc.tensor.matmul(
                            ps[j][:],
                            lhsT=w_s[:, ct, j * 128:(j + 1) * 128],
                            rhs=x2[:, half, :],
                            start=(ct == 0), stop=(ct == 7))
            dst = (pqts if tag == "qts" else pkts).tile(
                [128, 4, 512], bf16, tag=tag, name=tag)
            for j in range(4):
                nc.vector.tensor_copy(dst[:, j, :], ps[j][:])
            return dst

        def proj_v(t4, w_src=None):
            """v chunks: vext[kc][p=t%128, h, 0:64]=v, [..,64]=1."""
            ps = [pps.tile([128, 512], f32, tag="ps", name=f"ps{i}") for i in range(4)]
            for cp in range(4):
                if w_src is not None:
                    for h2 in range(2):
                        ct = 2 * cp + h2
                        nc.sync.dma_start(
                            wv_s[:, ct, :],
                            w_src[ct * 128:(ct + 1) * 128, :])
                x2 = load_x2(xv_d, cp, t4)
                for half in range(2):
                    ct = 2 * cp + half
                    for tc4 in range(4):
                        nc.tensor.matmul(
                            ps[tc4][:],
                            lhsT=x2[:, half, tc4 * 128:(tc4 + 1) * 128],
                            rhs=wv_s[:, ct, :],
                            start=(ct == 0), stop=(ct == 7))
            for tc4 in range(4):
                kc = 4 * t4 + tc4
                vx = pvext.tile([128, 8, 65], bf16, tag="vext", name="vx")
                nc.vector.tensor_copy(
                    vx[:, :, 0:64],
                    ps[tc4][:].rearrange("p (h d) -> p h d", h=8))
                nc.gpsimd.memset(vx[:, :, 64:65], 1.0)
                vext[kc] = vx

        def attention_hp(qt, qts, ctxn, hp):
            nki = 4 * qt + 4
            if True:
                ctx2 = [pps.tile([65, 512], f32, tag="ps", name=f"ctx{i}") for i in range(2)]
                pending = []   # ctx matmuls delayed one k-tile (SW pipeline)
                for ki in range(nki):
                    cur = []
                    for hh in range(2):
                        h = 2 * hp + hh
                        pb = hh * 64
                        diag = (ki // 4 == qt)
                        off = (ki % 4) * 128 if diag else 0
                        sT = pps.tile([128, 512], f32, tag="ps", name="sT")
                        ks = kts[ki // 4][pb:pb + 64, hp,
                                          (ki % 4) * 128:(ki % 4 + 1) * 128]
                        qs = qts[pb:pb + 64, hp, off:512]
                        nc.tensor.matmul(sT[:, off:], lhsT=ks, rhs=qs,
                                         start=True, stop=True)
                        ex = pex.tile([128, 512], bf16, tag="ex", name="ex")
                        nc.scalar.activation(ex[:, off:], sT[:, off:], EXP,
                                             scale=0.125)
                        if diag:
                            # zero the dead (k>q) triangle+cols of this block
                            nc.vector.tensor_mul(ex[:, off:off + 128],
                                                 ex[:, off:off + 128], cd[:])
                        cur.append((hh, h, off, ex, ki))
                    for (phh, ph, poff, pex_t, pki) in pending:
                        nc.tensor.matmul(
                            ctx2[phh][:, poff:], lhsT=vext[pki][:, ph, :],
                            rhs=pex_t[:, poff:],
                            start=(pki == 0), stop=(pki == nki - 1))
                    pending = cur
                for (phh, ph, poff, pex_t, pki) in pending:
                    nc.tensor.matmul(
                        ctx2[phh][:, poff:], lhsT=vext[pki][:, ph, :],
                        rhs=pex_t[:, poff:],
                        start=(pki == 0), stop=(pki == nki - 1))
                for hh in range(2):
                    srow = pr.tile([1, 512], f32, tag="srow", name="srow")
                    nc.vector.tensor_copy(srow[:], ctx2[hh][64:65, :])
                    rrow = pr.tile([1, 512], f32, tag="rrow", name="rrow")
                    nc.vector.reciprocal_approx_fast(rrow[:], srow[:])
                    rb = pr.tile([64, 512], f32, tag="rb", name="rb")
                    nc.gpsimd.partition_broadcast(rb[:], rrow[:])
                    if hh == 0:
                        nc.vector.tensor_mul(ctxn[0:64, hp, :],
                                             ctx2[hh][0:64, :], rb[:])
                    else:
                        tmp = pr.tile([64, 512], bf16, tag="tmp", name="tmp")
                        nc.vector.tensor_mul(tmp[:], ctx2[hh][0:64, :], rb[:])
                        nc.sync.dma_start(ctxn[64:128, hp, :], tmp[:])

        def outproj_chunk(qt, ctxn, qc4, ch, split=None):
            if split is None:
                yp = pps.tile([128, 512], f32, tag="ps", name="yp")
                js = range(4)
            else:
                yp = split
                js = (3,)
            for j in js:
                nc.tensor.matmul(
                    yp[:],
                    lhsT=ctxn[:, j, qc4 * 128:(qc4 + 1) * 128],
                    rhs=wo_s[:, j, ch * 512:(ch + 1) * 512],
                    start=(j == 0), stop=(j == 3))
            ys = pys.tile([128, 512], f32, tag="ys", name="ys")
            nc.vector.tensor_copy(ys[:], yp[:])
            nc.sync.dma_start(
                y_d[(qt * 4 + qc4) * 128:(qt * 4 + qc4 + 1) * 128,
                    ch * 512:(ch + 1) * 512], ys[:])


        # Interleave: attention(t4) hp-blocks carry next window's
        # projections and the previous window's out-projection on the
        # PE, keeping it dense (HAM warm) while ScalarE streams exps.
        # warm the ACT exp table during initial DMA
        warm = pr.tile([1, 8], f32, tag="warm", name="warm")
        nc.gpsimd.memset(warm[:], 0.0)
        nc.scalar.activation(warm[:], warm[:], EXP, scale=1.0)
        qts_cur = proj_qk(wq_s, xq_d, 0, "qts", w_src=wq_d)
        kts[0] = proj_qk(wk_s, xk_d, 0, "kts", w_src=wk_d)
        proj_v(0, w_src=wv_d)
        for j in range(4):
            nc.sync.dma_start(wo_s[:, j, :], wo_d[j * 128:(j + 1) * 128, :])
        prev_ctxn = None
        for t4 in range(4):
            ctxn = pctxn.tile([128, 4, 512], bf16, tag="ctxn", name="ctxn")
            qts_next = None
            for hp in range(4):
                attention_hp(t4, qts_cur, ctxn, hp)
                if t4 < 3:
                    if hp == 0:
                        qts_next = proj_qk(wq_s, xq_d, t4 + 1, "qts")
                    elif hp == 1:
                        kts[t4 + 1] = proj_qk(wk_s, xk_d, t4 + 1, "kts")
                    elif hp == 2:
                        proj_v(t4 + 1)
                if prev_ctxn is not None:
                    # spread the previous window's out-projection: 2 of
                    # its 8 chunks after each hp block
                    for c in range(2):
                        idx = hp * 2 + c
                        outproj_chunk(t4 - 1, prev_ctxn, idx // 2, idx % 2)
            prev_ctxn = ctxn
            qts_cur = qts_next
        for qc4 in range(4):
            for ch in range(2):
                outproj_chunk(3, prev_ctxn, qc4, ch)

    nc.compile()
    return nc


def _numpy_fallback(query, key, value, mask, causal_mask, Wq, Wk, Wv, Wo, bo):
    q = (query @ Wq.T).reshape(B, T, H, D).transpose(0, 2, 1, 3)
    k = (key @ Wk.T).reshape(B, T, H, D).transpose(0, 2, 1, 3)
    v = (value @ Wv.T).reshape(B, T, H, D).transpose(0, 2, 1, 3)
    out = np.zeros((B, H, T, D), np.float32)
    for b in range(B):
        for h in range(H):
            s = (q[b, h] @ k[b, h].T) / np.sqrt(np.float32(D))
            s = np.where(mask[b, 0, 0][None, :] == 0, -np.inf, s)
            if causal_mask:
                tri = np.tril(np.ones((T, T), bool))
                s = np.where(tri, s, -np.inf)
            s = s - s.max(axis=-1, keepdims=True)
            e = np.exp(s)
            a = e / e.sum(axis=-1, keepdims=True)
            out[b, h] = a @ v[b, h]
    out = out.transpose(0, 2, 1, 3).reshape(B, T, C)
    return out @ Wo.T + bo


def kernel(**inputs):
    from concourse import bass_utils

    inp = {k: np.asarray(v) for k, v in inputs.items()}
    query, key, value = inp["query"], inp["key"], inp["value"]
    Wq, Wk, Wv, Wo, bo = inp["Wq"], inp["Wk"], inp["Wv"], inp["Wo"], inp["bo"]
    mask, causal_mask = inp["mask"], int(inp["causal_mask"])

    if (mask == 0).any() or causal_mask != 1:
        return _numpy_fallback(
            query.astype(np.float32), key.astype(np.float32),
            value.astype(np.float32), mask, causal_mask,
            Wq.astype(np.float32), Wk.astype(np.float32),
            Wv.astype(np.float32), Wo.astype(np.float32),
            bo.astype(np.float32))

    if "nc" not in _CACHE:
        _CACHE["nc"] = _build()
    nc = _CACHE["nc"]

    cdiag = (np.arange(128)[:, None] <= np.arange(128)[None, :]
             ).astype(ml_dtypes.bfloat16)

    in_maps = []
    for core in range(8):
        b, g = core // 2, core % 2
        hs = g * HD
        in_maps.append({
            "xqT": np.ascontiguousarray(query[b].T).astype(ml_dtypes.bfloat16),
            "xkT": np.ascontiguousarray(key[b].T).astype(ml_dtypes.bfloat16),
            "xvT": np.ascontiguousarray(value[b].T).astype(ml_dtypes.bfloat16),
            "wq": np.ascontiguousarray(Wq[hs:hs + HD, :].T).astype(ml_dtypes.bfloat16),
            "wk": np.ascontiguousarray(Wk[hs:hs + HD, :].T).astype(ml_dtypes.bfloat16),
            "wv": np.ascontiguousarray(Wv[hs:hs + HD, :].T).astype(ml_dtypes.bfloat16),
            "wo": np.ascontiguousarray(Wo[:, hs:hs + HD].T).astype(ml_dtypes.bfloat16),
            "cdiag": cdiag,
        })

    res = bass_utils.run_bass_kernel_spmd(nc, in_maps, core_ids=list(range(8)))
    out = np.zeros((B, T, C), np.float32)
    for core in range(8):
        out[core // 2] += res.results[core]["y"]
    out += bo.astype(np.float32)
    return out


def run_traced(tmpdir=None, **inputs):
    """Profiled run (test harness helper): returns BassKernelResults with
    exec_time_ns/trace populated when the axon NTFF hook is available."""
    from concourse import bass_utils

    inp = {k: np.asarray(v) for k, v in inputs.items()}
    if "nc" not in _CACHE:
        _CACHE["nc"] = _build()
    nc = _CACHE["nc"]
    query, key, value = inp["query"], inp["key"], inp["value"]
    Wq, Wk, Wv, Wo = inp["Wq"], inp["Wk"], inp["Wv"], inp["Wo"]
    cdiag = (np.arange(128)[:, None] <= np.arange(128)[None, :]
             ).astype(ml_dtypes.bfloat16)
    in_maps = []
    for core in range(8):
        b, g = core // 2, core % 2
        hs = g * HD
        in_maps.append({
            "xqT": np.ascontiguousarray(query[b].T).astype(ml_dtypes.bfloat16),
            "xkT": np.ascontiguousarray(key[b].T).astype(ml_dtypes.bfloat16),
            "xvT": np.ascontiguousarray(value[b].T).astype(ml_dtypes.bfloat16),
            "wq": np.ascontiguousarray(Wq[hs:hs + HD, :].T).astype(ml_dtypes.bfloat16),
            "wk": np.ascontiguousarray(Wk[hs:hs + HD, :].T).astype(ml_dtypes.bfloat16),
            "wv": np.ascontiguousarray(Wv[hs:hs + HD, :].T).astype(ml_dtypes.bfloat16),
            "wo": np.ascontiguousarray(Wo[:, hs:hs + HD].T).astype(ml_dtypes.bfloat16),
            "cdiag": cdiag,
        })
    return bass_utils.run_bass_kernel_spmd(
        nc, in_maps, core_ids=list(range(8)), trace=True, tmpdir=tmpdir)
